# revision 15
# baseline (speedup 1.0000x reference)
"""Trainium2 Bass kernel for nn_CLIPModel_15006615734106 (GCN message passing).

Column-block sharding: core k computes sim[:, B_k] ([4096, 512]) which directly
serves as lhsT for the GCN aggregation matmuls. Analytic threshold
thr = S0.S1/(n0*n1). Collectives: AR (threshold partials), RS+AG (degrees),
AG (P1 = x@W1 row shards), AG (h@W2 row shards).

kernel(**inputs) -> (out [4096,1] f32, f_g [4096,818] f32,
                     loss_mask [4096,4096] bool, x [4096,768] f32)
"""

import os
import numpy as np

import concourse.bass as bass
import concourse.mybir as mybir
import concourse.tile as tile
from concourse import bacc, bass_utils
from concourse.masks import make_identity

N, C, H, O = 4096, 768, 512, 50
P = 8          # cores
R = N // P     # 512 rows per core
NCH = N // 128   # 32 i-chunks
CCH = C // 128   # 6 c-chunks
RCH = R // 128   # 4 r-chunks
HCH = H // 128   # 4 h-chunks

SIM_MODE = os.environ.get("SIM_MODE", "f32")  # "f32" | "f32r" | "bf16x3"

dt = mybir.dt
Alu = mybir.AluOpType
Act = mybir.ActivationFunctionType
Axis = mybir.AxisListType


def build(sim_mode=SIM_MODE):
    nc = bacc.Bacc("TRN2", target_bir_lowering=False, debug=False, num_devices=P)

    def din(name, shape, d=dt.float32):
        return nc.dram_tensor(name, shape, d, kind="ExternalInput").ap()

    def dout(name, shape, d=dt.float32):
        return nc.dram_tensor(name, shape, d, kind="ExternalOutput").ap()

    if sim_mode != "bf16x3":
        xT = din("xT", [C, N])             # full x transposed (replicated)
    xTb = din("xTb", [C, R])               # x.T columns of this core's block
    if sim_mode == "bf16x3":
        xT_hi = din("xT_hi", [C, N], dt.bfloat16)
        xT_lo = din("xT_lo", [C, N], dt.bfloat16)
        xTb_hi = din("xTb_hi", [C, R], dt.bfloat16)
        xTb_lo = din("xTb_lo", [C, R], dt.bfloat16)
    labB = din("labB", [128, R])           # labels[B_k] bcast over partitions
    lab_pc = din("lab_pc", [128, NCH])     # labels[128c+p] at (p,c)
    colidx = din("colidx", [128, R])       # r0+f bcast over partitions
    rowidx = din("rowidx", [128, NCH])     # 128c+p at (p,c)
    W1 = din("W1", [C, H])
    if sim_mode == "bf16x3":
        W1bf1 = din("W1bf", [C, H], dt.bfloat16)
        fcWxbf = din("fcWxbf", [C, 1], dt.bfloat16)
    W2bf = din("W2bf", [H, O], dt.bfloat16)
    b1_pc = din("b1_pc", [128, HCH])       # b1[128m+p] at (p,m)
    b2v = din("b2v", [1, O])
    fcWx = din("fcWx", [C, 1])
    wgv = din("wgv", [1, O])               # fcW[C:C+O, 0]
    fcbv = din("fcbv", [1, 1])

    lm_out = dout("lm_out", [N, R], dt.uint8)
    g_out = dout("g_out", [R, O])
    o_out = dout("o_out", [R, 1])

    mm_dt = {"f32": dt.float32, "f32r": dt.float32r}.get(sim_mode, dt.float32)

    with tile.TileContext(nc) as tc:
        with (
            tc.tile_pool(name="big", bufs=1) as big,      # xT/xTb/W tiles
            tc.tile_pool(name="apool", bufs=1) as apool,  # 32 adjacency tiles
            tc.tile_pool(name="sb", bufs=1) as sb,        # small working tiles
            tc.tile_pool(name="msk", bufs=1) as msk,      # sweep mask tiles
            tc.tile_pool(name="dram", bufs=1, space="DRAM") as dram,
        ):
            # ---------- load persistent tensors ----------
            xt = []   # lhsT tiles for sim (dtype per sim_mode)
            xtb = []  # rhs tiles for sim + lhsT for P1/fc (+ f32 view for prologue)
            for cc in range(CCH):
                if sim_mode != "bf16x3":
                    t = big.tile([128, N], mm_dt, tag=f"xt{cc}")
                    nc.sync.dma_start(t[:], xT[128 * cc:128 * (cc + 1), :].bitcast(mm_dt))
                    xt.append(t)
                tb = big.tile([128, R], mm_dt, tag=f"xtb{cc}")
                nc.sync.dma_start(tb[:], xTb[128 * cc:128 * (cc + 1), :].bitcast(mm_dt))
                xtb.append(tb)
            if sim_mode == "bf16x3":
                xth, xtl, xtbh, xtbl = [], [], [], []
                for cc in range(CCH):
                    sl = slice(128 * cc, 128 * (cc + 1))
                    t1 = big.tile([128, N], dt.bfloat16, tag=f"xth{cc}")
                    nc.sync.dma_start(t1[:], xT_hi[sl, :]); xth.append(t1)
                    t2 = big.tile([128, N], dt.bfloat16, tag=f"xtl{cc}")
                    nc.sync.dma_start(t2[:], xT_lo[sl, :]); xtl.append(t2)
                    t3 = big.tile([128, R], dt.bfloat16, tag=f"xtbh{cc}")
                    nc.sync.dma_start(t3[:], xTb_hi[sl, :]); xtbh.append(t3)
                    t4 = big.tile([128, R], dt.bfloat16, tag=f"xtbl{cc}")
                    nc.sync.dma_start(t4[:], xTb_lo[sl, :]); xtbl.append(t4)
            # dtype family for the P1/fc matmuls (lhsT comes from x-block tiles)
            w_dt = mm_dt if sim_mode != "bf16x3" else dt.bfloat16
            w1t = []
            for cc in range(CCH):
                t = big.tile([128, H], w_dt, tag=f"w1{cc}")
                nc.sync.dma_start(t[:], W1[128 * cc:128 * (cc + 1), :].bitcast(w_dt)
                                  if w_dt != dt.bfloat16 else W1bf1[128 * cc:128 * (cc + 1), :])
                w1t.append(t)
            fcwx = []
            for cc in range(CCH):
                t = big.tile([128, 1], w_dt, tag=f"fcwx{cc}")
                nc.sync.dma_start(t[:], fcWx[128 * cc:128 * (cc + 1), :].bitcast(w_dt)
                                  if w_dt != dt.bfloat16 else fcWxbf[128 * cc:128 * (cc + 1), :])
                fcwx.append(t)
            w2c = []
            for m in range(HCH):
                t = big.tile([128, O], dt.bfloat16, tag=f"w2{m}")
                nc.sync.dma_start(t[:], W2bf[128 * m:128 * (m + 1), :])
                w2c.append(t)
            labB_t = sb.tile([128, R], dt.float32, tag="labB")
            nc.sync.dma_start(labB_t[:], labB[:])
            lab_pc_t = sb.tile([128, NCH], dt.float32, tag="lab_pc")
            nc.sync.dma_start(lab_pc_t[:], lab_pc[:])
            colidx_t = sb.tile([128, R], dt.float32, tag="colidx")
            nc.sync.dma_start(colidx_t[:], colidx[:])
            rowidx_t = sb.tile([128, NCH], dt.float32, tag="rowidx")
            nc.sync.dma_start(rowidx_t[:], rowidx[:])
            b1_t = sb.tile([128, HCH], dt.float32, tag="b1")
            nc.sync.dma_start(b1_t[:], b1_pc[:])
            b2_t = sb.tile([1, O], dt.float32, tag="b2")
            nc.sync.dma_start(b2_t[:], b2v[:])
            wg_t = sb.tile([1, O], dt.float32, tag="wg")
            nc.sync.dma_start(wg_t[:], wgv[:])
            fcb_t = sb.tile([1, 1], dt.float32, tag="fcb")
            nc.sync.dma_start(fcb_t[:], fcbv[:])
            ones_t = sb.tile([128, 1], dt.float32, tag="ones")
            nc.vector.memset(ones_t[:], 1.0)
            ident = sb.tile([128, 128], dt.float32, tag="ident")
            make_identity(nc, ident[:])

            ps1 = tc.alloc_tile_pool(name="ps1", bufs=1, space="PSUM")

            def f32view(ap):
                return ap.bitcast(dt.float32) if mm_dt != dt.float32 else ap

            # ---------- prologue: analytic threshold ----------
            # partial u (colsum x) / v (label-masked colsum) over this block
            uv = sb.tile([128, 13], dt.float32, tag="uv")
            for cc in range(CCH):
                xv = f32view(xtb[cc][:])
                tmp = msk.tile([128, R], dt.float32, tag="ptmp", bufs=1)
                nc.any.tensor_tensor(tmp[:], xv, labB_t[:], Alu.mult)
                nc.vector.tensor_reduce(uv[:, 6 + cc:7 + cc], tmp[:], Axis.X, Alu.add)
                nc.vector.tensor_reduce(uv[:, cc:cc + 1], xv, Axis.X, Alu.add)
            nc.vector.tensor_reduce(uv[:, 12:13], labB_t[:], Axis.X, Alu.add)
            uv_in = dram.tile([128, 13], dt.float32)
            uv_out = dram.tile([128, 13], dt.float32, addr_space="Shared")
            nc.sync.dma_start(uv_in[:], uv[:])
            nc.gpsimd.collective_compute(
                "AllReduce", Alu.add, replica_groups=[list(range(P))],
                ins=[uv_in[:].opt()], outs=[uv_out[:].opt()])
            uvf = sb.tile([128, 13], dt.float32, tag="uvf")
            nc.sync.dma_start(uvf[:], uv_out[:])
            s0 = sb.tile([128, 6], dt.float32, tag="s0")
            nc.any.tensor_tensor(s0[:], uvf[:, 0:6], uvf[:, 6:12], Alu.subtract)
            st = sb.tile([128, 6], dt.float32, tag="st")
            nc.any.tensor_tensor(st[:], s0[:], uvf[:, 6:12], Alu.mult)
            rowred = sb.tile([128, 1], dt.float32, tag="rowred")
            nc.vector.tensor_reduce(rowred[:], st[:], Axis.X, Alu.add)
            s_ps = ps1.tile([1, 1], dt.float32, tag="sc", bufs=1)
            nc.tensor.matmul(s_ps[:], rowred[:], ones_t[:], start=True, stop=True)
            n1_ps = ps1.tile([1, 1], dt.float32, tag="sc", bufs=1)
            nc.tensor.matmul(n1_ps[:], uvf[:, 12:13], ones_t[:], start=True, stop=True)
            s_sb = sb.tile([1, 1], dt.float32, tag="s_sb")
            nc.vector.tensor_copy(s_sb[:], s_ps[:])
            n1_sb = sb.tile([1, 1], dt.float32, tag="n1_sb")
            nc.vector.tensor_copy(n1_sb[:], n1_ps[:])
            n0_sb = sb.tile([1, 1], dt.float32, tag="n0_sb")
            nc.vector.tensor_scalar(n0_sb[:], n1_sb[:], -1.0, float(N), Alu.mult, Alu.add)
            nn_sb = sb.tile([1, 1], dt.float32, tag="nn_sb")
            nc.any.tensor_tensor(nn_sb[:], n0_sb[:], n1_sb[:], Alu.mult)
            rcp = sb.tile([1, 1], dt.float32, tag="rcp")
            nc.vector.reciprocal(rcp[:], nn_sb[:])
            thr = sb.tile([1, 1], dt.float32, tag="thr")
            nc.any.tensor_tensor(thr[:], s_sb[:], rcp[:], Alu.mult)
            thr_b = sb.tile([128, 1], dt.float32, tag="thr_b")
            nc.gpsimd.partition_broadcast(thr_b[:], thr[:])

            # ---------- sim colblock + mask sweep ----------
            a_tiles = []
            deg_pc = sb.tile([128, NCH], dt.float32, tag="deg_pc")
            for c in range(NCH):
                isl = slice(128 * c, 128 * (c + 1))
                sim_ps = ps1.tile([128, R], dt.float32, tag="psim", bufs=2)
                if sim_mode == "bf16x3":
                    for cc in range(CCH):
                        nc.tensor.matmul(sim_ps[:], xth[cc][:, isl], xtbh[cc][:],
                                         start=(cc == 0), stop=False)
                        nc.tensor.matmul(sim_ps[:], xth[cc][:, isl], xtbl[cc][:],
                                         start=False, stop=False)
                        nc.tensor.matmul(sim_ps[:], xtl[cc][:, isl], xtbh[cc][:],
                                         start=False, stop=(cc == CCH - 1))
                else:
                    for cc in range(CCH):
                        nc.tensor.matmul(sim_ps[:], xt[cc][:, isl], xtb[cc][:],
                                         start=(cc == 0), stop=(cc == CCH - 1))
                t_m = msk.tile([128, R], dt.bfloat16, tag="t", bufs=2)
                nc.vector.tensor_scalar(t_m[:], sim_ps[:], thr_b[:], None, Alu.is_le)
                same = msk.tile([128, R], dt.bfloat16, tag="same", bufs=2)
                nc.any.tensor_scalar(same[:], labB_t[:], lab_pc_t[:, c:c + 1], None, Alu.is_equal)
                a0 = msk.tile([128, R], dt.bfloat16, tag="a0", bufs=2)
                nc.any.tensor_tensor(a0[:], t_m[:], same[:], Alu.mult)
                ltm = msk.tile([128, R], dt.bfloat16, tag="ltm", bufs=2)
                nc.any.tensor_scalar(ltm[:], colidx_t[:], rowidx_t[:, c:c + 1], None, Alu.is_gt)
                lmt = msk.tile([128, R], dt.uint8, tag="lm", bufs=2)
                nc.any.tensor_tensor(lmt[:], a0[:], ltm[:], Alu.mult)
                nc.sync.dma_start(lm_out[isl, :], lmt[:])
                eyem = msk.tile([128, R], dt.bfloat16, tag="eye", bufs=2)
                nc.any.tensor_scalar(eyem[:], colidx_t[:], rowidx_t[:, c:c + 1], None, Alu.is_equal)
                a_c = apool.tile([128, R], dt.bfloat16, tag="a", bufs=NCH)
                nc.any.tensor_tensor(a_c[:], a0[:], eyem[:], Alu.add)
                nc.vector.tensor_reduce(deg_pc[:, c:c + 1], a_c[:], Axis.X, Alu.add)
                a_tiles.append(a_c)

            # ---------- P1 = x@W1 row shard + AllGather ----------
            p1_in = dram.tile([R, H], dt.float32)
            p1_full = dram.tile([N, H], dt.float32, addr_space="Shared")
            px = xtb if sim_mode != "bf16x3" else xtbh
            for rr in range(RCH):
                p1ps = ps1.tile([128, H], dt.float32, tag="p1b", bufs=2)
                for cc in range(CCH):
                    nc.tensor.matmul(p1ps[:], px[cc][:, 128 * rr:128 * (rr + 1)], w1t[cc][:],
                                     start=(cc == 0), stop=(cc == CCH - 1))
                p1s = sb.tile([128, H], dt.float32, tag="p1s", bufs=2)
                nc.any.tensor_copy(p1s[:], p1ps[:])
                nc.sync.dma_start(p1_in[128 * rr:128 * (rr + 1), :], p1s[:])
            nc.gpsimd.collective_compute(
                "AllGather", Alu.bypass, replica_groups=[list(range(P))],
                ins=[p1_in[:].opt()], outs=[p1_full[:].opt()])

            # ---------- degrees: transpose -> RS -> rsqrt -> AG ----------
            degT_ps = ps1.tile([NCH, 128], dt.float32, tag="tp", bufs=2)
            nc.tensor.transpose(degT_ps[:], deg_pc[:], ident[:])
            degT = sb.tile([NCH, 128], dt.float32, tag="degT")
            nc.vector.tensor_copy(degT[:], degT_ps[:])
            deg_rs_in = dram.tile([NCH, 128], dt.float32)
            deg_rs_out = dram.tile([RCH, 128], dt.float32)
            nc.sync.dma_start(deg_rs_in[:], degT[:])
            nc.gpsimd.collective_compute(
                "ReduceScatter", Alu.add, replica_groups=[list(range(P))],
                ins=[deg_rs_in[:].opt()], outs=[deg_rs_out[:].opt()])
            degB = sb.tile([1, R], dt.float32, tag="degB")
            nc.sync.dma_start(degB[:], deg_rs_out[:].rearrange("a b -> (a b)"))
            sqv = sb.tile([1, R], dt.float32, tag="sqv")
            nc.scalar.sqrt(sqv[:], degB[:])
            dinvr = sb.tile([1, R], dt.float32, tag="dinvr")
            nc.vector.reciprocal(dinvr[:], sqv[:])
            dinvr_b = sb.tile([128, R], dt.float32, tag="dinvr_b")
            nc.gpsimd.partition_broadcast(dinvr_b[:], dinvr[:])
            dinvr_bb = sb.tile([128, R], dt.bfloat16, tag="dinvr_bb")
            nc.vector.tensor_copy(dinvr_bb[:], dinvr_b[:])
            dinv_ag_in = dram.tile([RCH, 128], dt.float32)
            dinv_ag_out = dram.tile([NCH, 128], dt.float32, addr_space="Shared")
            nc.sync.dma_start(dinv_ag_in[:].rearrange("a b -> (a b)"), dinvr[:])
            nc.gpsimd.collective_compute(
                "AllGather", Alu.bypass, replica_groups=[list(range(P))],
                ins=[dinv_ag_in[:].opt()], outs=[dinv_ag_out[:].opt()])
            dinvT = sb.tile([NCH, 128], dt.float32, tag="dinvT")
            nc.sync.dma_start(dinvT[:], dinv_ag_out[:])
            dinv_ps = ps1.tile([128, NCH], dt.float32, tag="tp", bufs=2)
            nc.tensor.transpose(dinv_ps[:], dinvT[:], ident[0:NCH, 0:NCH])
            dinv_pc = sb.tile([128, NCH], dt.float32, tag="dinv_pc")
            nc.vector.tensor_copy(dinv_pc[:], dinv_ps[:])

            # scale adjacency columns by dinv_r in place: a' = a * dinv_r
            for c in range(NCH):
                nc.any.tensor_tensor(a_tiles[c][:], a_tiles[c][:], dinvr_bb[:], Alu.mult)

            ps1.release()
            ps2 = tc.alloc_tile_pool(name="ps2", bufs=1, space="PSUM")

            # ---------- AGG1: hT = (D P1)^T @ a' ----------
            hts = [ps2.tile([128, R], dt.float32, tag="hts", bufs=HCH, name=f"hts{m}")
                   for m in range(HCH)]
            for c in range(NCH):
                isl = slice(128 * c, 128 * (c + 1))
                dp1 = sb.tile([128, H], dt.float32, tag="dp1", bufs=3)
                nc.sync.dma_start(dp1[:], p1_full[isl, :])
                dp1b = sb.tile([128, H], dt.bfloat16, tag="dp1b", bufs=3)
                nc.any.tensor_scalar(dp1b[:], dp1[:], dinv_pc[:, c:c + 1], None, Alu.mult)
                for m in range(HCH):
                    nc.tensor.matmul(hts[m][:], dp1b[:, 128 * m:128 * (m + 1)], a_tiles[c][:],
                                     start=(c == 0), stop=(c == NCH - 1))
            h_m = []
            for m in range(HCH):
                hm = sb.tile([128, R], dt.bfloat16, tag=f"h{m}")
                nc.scalar.activation(hm[:], hts[m][:], Act.Relu, bias=b1_t[:, m:m + 1], scale=1.0)
                h_m.append(hm)

            # ---------- hw2 = h @ W2 row shard + AllGather ----------
            hw2_in = dram.tile([R, O], dt.float32)
            hw2_full = dram.tile([N, O], dt.float32, addr_space="Shared")
            for rr in range(RCH):
                hwps = ps2.tile([128, O], dt.float32, tag="hw", bufs=2)
                for m in range(HCH):
                    nc.tensor.matmul(hwps[:], h_m[m][:, 128 * rr:128 * (rr + 1)], w2c[m][:],
                                     start=(m == 0), stop=(m == HCH - 1))
                hws = sb.tile([128, O], dt.float32, tag="hws", bufs=2)
                nc.any.tensor_copy(hws[:], hwps[:])
                nc.sync.dma_start(hw2_in[128 * rr:128 * (rr + 1), :], hws[:])
            nc.gpsimd.collective_compute(
                "AllGather", Alu.bypass, replica_groups=[list(range(P))],
                ins=[hw2_in[:].opt()], outs=[hw2_full[:].opt()])

            ps2.release()
            ps3 = tc.alloc_tile_pool(name="ps3", bufs=1, space="PSUM")

            # ---------- AGG2: g = a'^T @ (D hw2) + b2 ----------
            gps = [ps3.tile([128, O], dt.float32, tag="gps", bufs=RCH, name=f"gps{m}")
                   for m in range(RCH)]
            for c in range(NCH):
                isl = slice(128 * c, 128 * (c + 1))
                dhw = sb.tile([128, O], dt.float32, tag="dhw", bufs=3)
                nc.sync.dma_start(dhw[:], hw2_full[isl, :])
                dhwb = sb.tile([128, O], dt.bfloat16, tag="dhwb", bufs=3)
                nc.any.tensor_scalar(dhwb[:], dhw[:], dinv_pc[:, c:c + 1], None, Alu.mult)
                for m in range(RCH):
                    nc.tensor.matmul(gps[m][:], a_tiles[c][:, 128 * m:128 * (m + 1)], dhwb[:],
                                     start=(c == 0), stop=(c == NCH - 1))
            b2_b = sb.tile([128, O], dt.float32, tag="b2_b")
            nc.gpsimd.partition_broadcast(b2_b[:], b2_t[:])
            wg_b = sb.tile([128, O], dt.float32, tag="wg_b")
            nc.gpsimd.partition_broadcast(wg_b[:], wg_t[:])
            fcb_b = sb.tile([128, 1], dt.float32, tag="fcb_b")
            nc.gpsimd.partition_broadcast(fcb_b[:], fcb_t[:])

            # ---------- g, fc out ----------
            for m in range(RCH):
                rsl = slice(128 * m, 128 * (m + 1))
                gf = sb.tile([128, O], dt.float32, tag="gf", bufs=2)
                nc.any.tensor_tensor(gf[:], gps[m][:], b2_b[:], Alu.add)
                nc.sync.dma_start(g_out[rsl, :], gf[:])
                gw = sb.tile([128, O], dt.float32, tag="gw", bufs=2)
                nc.any.tensor_tensor(gw[:], gf[:], wg_b[:], Alu.mult)
                gwr = sb.tile([128, 1], dt.float32, tag="gwr", bufs=2)
                nc.vector.tensor_reduce(gwr[:], gw[:], Axis.X, Alu.add)
                fxps = ps3.tile([128, 1], dt.float32, tag="fx", bufs=2)
                for cc in range(CCH):
                    nc.tensor.matmul(fxps[:], px[cc][:, rsl], fcwx[cc][:],
                                     start=(cc == 0), stop=(cc == CCH - 1))
                o1 = sb.tile([128, 1], dt.float32, tag="o1", bufs=2)
                nc.any.tensor_tensor(o1[:], fxps[:], gwr[:], Alu.add)
                o2 = sb.tile([128, 1], dt.float32, tag="o2", bufs=2)
                nc.any.tensor_tensor(o2[:], o1[:], fcb_b[:], Alu.add)
                nc.sync.dma_start(o_out[rsl, :], o2[:])
            ps3.release()

    nc.compile()
    return nc


_nc_cache = {}


def get_nc(sim_mode=SIM_MODE):
    if sim_mode not in _nc_cache:
        _nc_cache[sim_mode] = build(sim_mode)
    return _nc_cache[sim_mode]


def prep_in_maps(x, labels, W1, b1, W2, b2, fcW, fcb, sim_mode=SIM_MODE):
    import ml_dtypes
    x = np.ascontiguousarray(np.asarray(x, dtype=np.float32))
    lab = np.asarray(labels).astype(np.float32)
    xT = np.ascontiguousarray(x.T)
    W1 = np.ascontiguousarray(np.asarray(W1, dtype=np.float32))
    b1 = np.asarray(b1, dtype=np.float32)
    W2bf = np.asarray(W2, dtype=np.float32).astype(ml_dtypes.bfloat16)
    b2 = np.asarray(b2, dtype=np.float32)
    fcW = np.asarray(fcW, dtype=np.float32)
    fcb = np.asarray(fcb, dtype=np.float32)

    lab_pc = np.ascontiguousarray(lab.reshape(NCH, 128).T)
    rowidx = np.ascontiguousarray(
        np.arange(N, dtype=np.float32).reshape(NCH, 128).T)
    b1_pc = np.ascontiguousarray(b1.reshape(HCH, 128).T)
    fcWx = np.ascontiguousarray(fcW[:C, 0:1])
    wgv = np.ascontiguousarray(fcW[C:, 0]).reshape(1, O)
    fcbv = fcb.reshape(1, 1)

    if sim_mode == "bf16x3":
        xT_hi = np.ascontiguousarray(xT.astype(ml_dtypes.bfloat16))
        xT_lo = np.ascontiguousarray(
            (xT - xT_hi.astype(np.float32)).astype(ml_dtypes.bfloat16))

    in_maps = []
    for k in range(P):
        blk = slice(R * k, R * (k + 1))
        m = {
            "xTb": np.ascontiguousarray(xT[:, blk]),
            "labB": np.ascontiguousarray(np.broadcast_to(lab[blk], (128, R))),
            "lab_pc": lab_pc,
            "colidx": np.ascontiguousarray(np.broadcast_to(
                np.arange(R * k, R * (k + 1), dtype=np.float32), (128, R))),
            "rowidx": rowidx,
            "W1": W1,
            "W2bf": W2bf,
            "b1_pc": b1_pc,
            "b2v": b2.reshape(1, O),
            "fcWx": fcWx,
            "wgv": wgv,
            "fcbv": fcbv,
        }
        if sim_mode == "bf16x3":
            m["xT_hi"] = xT_hi
            m["xT_lo"] = xT_lo
            m["xTb_hi"] = np.ascontiguousarray(xT_hi[:, blk])
            m["xTb_lo"] = np.ascontiguousarray(xT_lo[:, blk])
            m["W1bf"] = W1.astype(ml_dtypes.bfloat16)
            m["fcWxbf"] = fcWx.astype(ml_dtypes.bfloat16)
        else:
            m["xT"] = xT
        in_maps.append(m)
    return in_maps


def run(inputs, sim_mode=SIM_MODE, trace=False, **kw):
    nc = get_nc(sim_mode)
    in_maps = prep_in_maps(
        inputs["x"], inputs["labels"], inputs["W1"], inputs["b1"],
        inputs["W2"], inputs["b2"], inputs["fcW"], inputs["fcb"],
        sim_mode=sim_mode)
    res = bass_utils.run_bass_kernel_spmd(
        nc, in_maps, core_ids=list(range(P)), trace=trace, **kw)
    x = np.asarray(inputs["x"], dtype=np.float32)
    lm = np.concatenate([res.results[k]["lm_out"] for k in range(P)], axis=1)
    g = np.concatenate([res.results[k]["g_out"] for k in range(P)], axis=0)
    o = np.concatenate([res.results[k]["o_out"] for k in range(P)], axis=0)
    f_g = np.concatenate([x, g], axis=1)
    return (o, f_g, lm.astype(bool), x), res


def kernel(**inputs):
    outs, _ = run(inputs)
    return outs
